# revision 1
# baseline (speedup 1.0000x reference)
"""Trainium2 Bass kernel for nn_AttentiveEncoderPOS (embed+concat+linear+self-attention).

Strategy (8 cores, SPMD, no collectives):
  - Each core receives input_ids/pos_ids ROTATED so that its 1024-row slice
    comes first. Softmax attention is invariant to key/value permutation, so
    each core computes the full L = concat(emb[ids], pos[pids]) @ W.T + b
    (keys/values, in its own order) and attends only its first 1024 rows
    (queries) against all 8192 keys. Output rows i*1024:(i+1)*1024 come from
    core i directly.
  - Layout: L is produced TRANSPOSED (L.T, h on partitions) by the linear
    matmul; scores are computed transposed (keys on partitions, q on free) so
    that exp(scores) feeds the A@V matmul directly as the stationary operand
    and the softmax denominator is a per-partition scale after A@V.
  - bf16 matmul inputs, fp32 PSUM accumulation. Scores are tiny (|s|<0.01)
    so exp() without max-subtraction is exact softmax.
"""

import numpy as np

import concourse.bass as bass
import concourse.mybir as mybir
from concourse import bacc
from concourse.tile import TileContext
from concourse.bass_utils import run_bass_kernel_spmd
from concourse.masks import make_identity

N = 8192
H = 1024
VOCAB = 50257
POS = 64
NCORES = 8
NL = N // NCORES          # 1024 query rows per core
P = 128
KT = N // P               # 64 key tiles
HT = H // P               # 8 h tiles
CHUNK = 512
NCH = N // CHUNK          # 16 phase-1 chunks
RT = CHUNK // P           # 4 row tiles / chunk
K2 = 2 * H
KTI = K2 // P             # 16 contraction tiles for the linear
QTN = NL // P             # 8 q tiles
BLK = 8                   # key tiles per phase-2 block (PSUM accum chain len)
NBLK = KT // BLK
SCALE = 1.0 / 32.0        # 1/sqrt(H)

BF = mybir.dt.bfloat16
F32 = mybir.dt.float32
I32 = mybir.dt.int32
EXP = mybir.ActivationFunctionType.Exp


def build_nc():
    nc = bacc.Bacc()
    ids = nc.declare_dram_parameter("ids", [KT, P, 1], I32, isOutput=False)
    pids = nc.declare_dram_parameter("pids", [KT, P, 1], I32, isOutput=False)
    emb = nc.declare_dram_parameter("emb", [VOCAB, H], F32, isOutput=False)
    pemb = nc.declare_dram_parameter("pemb", [POS, H], F32, isOutput=False)
    wt = nc.declare_dram_parameter("wt", [K2, H], F32, isOutput=False)  # W.T
    bias = nc.declare_dram_parameter("bias", [HT, P, 1], F32, isOutput=False)
    out = nc.declare_dram_parameter("out", [NL, H], F32, isOutput=True)

    # L.T tile-blocked: [key-tile][h-tile][128 h, 128 key] bf16
    lt_d = nc.dram_tensor("lt_d", [KT, HT, P, P], BF)
    # V (= L, natural layout): [key-tile][128 key, 1024 h] bf16
    v_d = nc.dram_tensor("v_d", [KT, P, H], BF)

    with TileContext(nc) as tc:
        with tc.tile_pool(name="const", bufs=1) as const:
            ident = const.tile([P, P], BF)
            make_identity(nc, ident[:])
            ones = const.tile([P, 1], BF)
            nc.gpsimd.memset(ones[:], 1.0)
            ident32 = const.tile([P, P], F32)
            make_identity(nc, ident32[:])
            b_row = const.tile([1, H], F32)
            nc.sync.dma_start(
                out=b_row[0:1, :], in_=bias.rearrange("h p u -> u (h p)")
            )
            b_sb = const.tile([P, HT], F32)
            nc.sync.dma_start(
                out=b_sb[:].rearrange("p (h u) -> p h u", h=HT),
                in_=bias.rearrange("h p u -> p h u"),
            )

            # ---------------- Phase 1: L.T and V production ----------------
            with (
                tc.tile_pool(name="wtp", bufs=KTI) as wtp,
                tc.tile_pool(name="wld", bufs=2) as wld,
                tc.tile_pool(name="idp", bufs=8) as idp,
                tc.tile_pool(name="xfp", bufs=3) as xfp,
                tc.tile_pool(name="xbp", bufs=RT + 2) as xbp,
                tc.tile_pool(name="xtp", bufs=2 * KTI) as xtp,
                tc.tile_pool(name="ltp", bufs=2 * HT) as ltp,
                tc.tile_pool(name="lup", bufs=2 * HT) as lup,
                tc.tile_pool(name="vp", bufs=8) as vp,
                tc.tile_pool(name="tps", bufs=3, space="PSUM") as tps,
                tc.tile_pool(name="mps", bufs=2, space="PSUM") as mps,
            ):
                # W.T -> bf16 SBUF, one [128, H] tile per contraction k-tile
                wtb = []
                for k in range(KTI):
                    wf = wld.tile([P, H], F32, tag="wld")
                    nc.sync.dma_start(out=wf[:], in_=wt[k * P : (k + 1) * P, :])
                    wb = wtp.tile([P, H], BF, tag="wtb")
                    nc.vector.tensor_copy(out=wb[:], in_=wf[:])
                    wtb.append(wb)

                for ch in range(NCH):
                    # gather + transpose X for this chunk of 512 rows
                    xts = []
                    for k in range(KTI):
                        xts.append(xtp.tile([P, CHUNK], BF, tag="xt", name="xt"))
                    xbs = []
                    for rt in range(RT):
                        t = ch * RT + rt
                        idt = idp.tile([P, 1], I32, tag="id")
                        nc.sync.dma_start(out=idt[:], in_=ids[t])
                        pidt = idp.tile([P, 1], I32, tag="pid")
                        nc.sync.dma_start(out=pidt[:], in_=pids[t])
                        xf = xfp.tile([P, K2], F32, tag="xf")
                        nc.gpsimd.indirect_dma_start(
                            out=xf[:, 0:H],
                            out_offset=None,
                            in_=emb[:],
                            in_offset=bass.IndirectOffsetOnAxis(ap=idt[:, :1], axis=0),
                        )
                        nc.gpsimd.indirect_dma_start(
                            out=xf[:, H:K2],
                            out_offset=None,
                            in_=pemb[:],
                            in_offset=bass.IndirectOffsetOnAxis(ap=pidt[:, :1], axis=0),
                        )
                        xb = xbp.tile([P, K2], BF, tag="xb")
                        nc.vector.tensor_copy(out=xb[:], in_=xf[:])
                        xbs.append(xb)
                    for k in range(KTI):
                        pt = tps.tile([P, CHUNK], BF, tag="tp")
                        for rt in range(RT):
                            nc.tensor.transpose(
                                pt[:, rt * P : (rt + 1) * P],
                                xbs[rt][:, k * P : (k + 1) * P],
                                ident[:],
                            )
                        nc.vector.tensor_copy(out=xts[k][:], in_=pt[:])

                    # linear: L.T[ht, chunk] = sum_k W.T[k,ht].T @ X.T[k,chunk]
                    lts = []
                    for ht in range(HT):
                        pm = mps.tile([P, CHUNK], F32, tag="mp")
                        for k in range(KTI):
                            nc.tensor.matmul(
                                pm[:],
                                lhsT=wtb[k][:, ht * P : (ht + 1) * P],
                                rhs=xts[k][:],
                                start=(k == 0),
                                stop=(k == KTI - 1),
                            )
                        lt = ltp.tile([P, CHUNK], BF, tag="lt")
                        nc.vector.tensor_add(
                            out=lt[:],
                            in0=pm[:],
                            in1=b_sb[:, ht : ht + 1].to_broadcast([P, CHUNK]),
                        )
                        ltu = lup.tile([P, CHUNK], BF, tag="ltu")
                        nc.vector.tensor_copy(out=ltu[:], in_=pm[:])
                        lts.append(ltu)
                        nc.sync.dma_start(
                            out=lt_d[ch * RT : (ch + 1) * RT, ht].rearrange(
                                "kb p c -> p kb c"
                            ),
                            in_=lt[:].rearrange("p (kb c) -> p kb c", kb=RT),
                        )
                    # V tiles: transpose L.T chunk back to natural layout
                    for kb in range(RT):
                        vt = vp.tile([P, H], BF, tag="v")
                        pt = tps.tile([P, H], BF, tag="tpv")
                        for ht in range(HT):
                            nc.tensor.transpose(
                                pt[:, ht * P : (ht + 1) * P],
                                lts[ht][:, kb * P : (kb + 1) * P],
                                ident[:],
                            )
                        nc.vector.tensor_copy(out=vt[:], in_=pt[:])
                        nc.sync.dma_start(out=v_d[ch * RT + kb], in_=vt[:])

            # ---------------- Phase 2: attention ----------------
            with (
                tc.tile_pool(name="qtp", bufs=HT) as qtp,
                tc.tile_pool(name="op", bufs=QTN) as op,
                tc.tile_pool(name="lkp", bufs=4) as lkp,
                tc.tile_pool(name="ep", bufs=2 * BLK) as ep,
                tc.tile_pool(name="vp2", bufs=2 * BLK) as vp2,
                tc.tile_pool(name="fin", bufs=2) as fin,
                tc.tile_pool(name="sps", bufs=2, space="PSUM") as sps,
                tc.tile_pool(name="ops", bufs=2, space="PSUM") as ops,
                tc.tile_pool(name="cps", bufs=1, space="PSUM") as cps,
            ):
                # Q.T resident: first NL columns of L.T
                qts = []
                for ht in range(HT):
                    q = qtp.tile([P, NL], BF, tag="qt")
                    nc.sync.dma_start(
                        out=q[:].rearrange("p (k c) -> p k c", k=QTN),
                        in_=lt_d[0:QTN, ht].rearrange("k p c -> p k c"),
                    )
                    qts.append(q)

                psum_c = cps.tile([P, QTN], F32, tag="cs")
                out_sb = []
                for qt in range(QTN):
                    out_sb.append(op.tile([P, H], F32, tag="o", name="o"))

                for blk in range(NBLK):
                    es = []
                    vts = []
                    for j in range(BLK):
                        kt = blk * BLK + j
                        ltk = lkp.tile([P, H], BF, tag="lk")
                        nc.sync.dma_start(
                            out=ltk[:].rearrange("p (h c) -> p h c", h=HT),
                            in_=lt_d[kt].rearrange("h p c -> p h c"),
                        )
                        e = ep.tile([P, NL], BF, tag="e")
                        for qc in range(NL // CHUNK):
                            ps = sps.tile([P, CHUNK], F32, tag="sp")
                            for ht in range(HT):
                                nc.tensor.matmul(
                                    ps[:],
                                    lhsT=ltk[:, ht * P : (ht + 1) * P],
                                    rhs=qts[ht][:, qc * CHUNK : (qc + 1) * CHUNK],
                                    start=(ht == 0),
                                    stop=(ht == HT - 1),
                                )
                            nc.scalar.activation(
                                out=e[:, qc * CHUNK : (qc + 1) * CHUNK],
                                in_=ps[:],
                                func=EXP,
                                scale=SCALE,
                            )
                        es.append(e)
                        # colsum accumulation (denominator), one chain per q tile
                        for qt in range(QTN):
                            nc.tensor.matmul(
                                psum_c[:, qt : qt + 1],
                                lhsT=e[:, qt * P : (qt + 1) * P],
                                rhs=ones[:],
                                start=(kt == 0),
                                stop=(kt == KT - 1),
                            )
                        vt = vp2.tile([P, H], BF, tag="v2")
                        nc.sync.dma_start(out=vt[:], in_=v_d[kt])
                        vts.append(vt)

                    last_blk = blk == NBLK - 1
                    if last_blk:
                        # colsum is complete: build its row layout for the
                        # exact rank-1 bias term colsum[q] * b[h]
                        cs_sb = fin.tile([P, QTN], F32, tag="cs_sb")
                        nc.vector.tensor_copy(out=cs_sb[:], in_=psum_c[:])
                        cs_row = fin.tile([1, NL], F32, tag="cs_row")
                        for qt in range(QTN):
                            cs_tp = sps.tile([1, P], F32, tag="ct", bufs=1)
                            nc.tensor.transpose(
                                cs_tp[:], cs_sb[:, qt : qt + 1], ident32[:]
                            )
                            nc.vector.tensor_copy(
                                out=cs_row[0:1, qt * P : (qt + 1) * P], in_=cs_tp[:]
                            )
                    for qt in range(QTN):
                        po = ops.tile([P, H], F32, tag="op")
                        for j in range(BLK):
                            for hh in range(H // CHUNK):
                                nc.tensor.matmul(
                                    po[:, hh * CHUNK : (hh + 1) * CHUNK],
                                    lhsT=es[j][:, qt * P : (qt + 1) * P],
                                    rhs=vts[j][:, hh * CHUNK : (hh + 1) * CHUNK],
                                    start=(j == 0),
                                    stop=(j == BLK - 1 and not last_blk),
                                )
                        if last_blk:
                            for hh in range(H // CHUNK):
                                nc.tensor.matmul(
                                    po[:, hh * CHUNK : (hh + 1) * CHUNK],
                                    lhsT=cs_row[0:1, qt * P : (qt + 1) * P],
                                    rhs=b_row[0:1, hh * CHUNK : (hh + 1) * CHUNK],
                                    start=False,
                                    stop=True,
                                )
                        if blk == 0:
                            nc.vector.tensor_copy(out=out_sb[qt][:], in_=po[:])
                        else:
                            nc.vector.tensor_add(
                                out=out_sb[qt][:], in0=out_sb[qt][:], in1=po[:]
                            )

                rec = fin.tile([P, QTN], F32, tag="rec")
                nc.vector.reciprocal(rec[:], psum_c[:])
                for qt in range(QTN):
                    nc.vector.tensor_mul(
                        out=out_sb[qt][:],
                        in0=out_sb[qt][:],
                        in1=rec[:, qt : qt + 1].to_broadcast([P, H]),
                    )
                    nc.sync.dma_start(
                        out=out[qt * P : (qt + 1) * P, :], in_=out_sb[qt][:]
                    )
    nc.finalize()
    return nc


def _prep_inputs(inputs):
    ids = np.asarray(inputs["input_ids"]).astype(np.int32)
    pids = np.asarray(inputs["pos_ids"]).astype(np.int32)
    emb = np.asarray(inputs["emb"], dtype=np.float32)
    pemb = np.asarray(inputs["pos_emb"], dtype=np.float32)
    W = np.asarray(inputs["W"], dtype=np.float32)
    b = np.asarray(inputs["b"], dtype=np.float32)
    wt = np.ascontiguousarray(W.T)                      # [2H, H]
    bias = np.ascontiguousarray(b.reshape(HT, P, 1))
    in_maps = []
    for i in range(NCORES):
        r = np.roll(ids, -NL * i)
        rp = np.roll(pids, -NL * i)
        in_maps.append(
            {
                "ids": np.ascontiguousarray(r.reshape(KT, P, 1)),
                "pids": np.ascontiguousarray(rp.reshape(KT, P, 1)),
                "emb": emb,
                "pemb": pemb,
                "wt": wt,
                "bias": bias,
            }
        )
    return in_maps


def run(inputs, trace=False):
    nc = build_nc()
    in_maps = _prep_inputs(inputs)
    res = run_bass_kernel_spmd(nc, in_maps, list(range(NCORES)), trace=trace)
    out = np.concatenate([res.results[i]["out"] for i in range(NCORES)], axis=0)
    return out, res


def kernel(**inputs):
    out, _ = run(inputs, trace=False)
    return out



# revision 2
# speedup vs baseline: 1.4911x; 1.4911x over previous
"""Trainium2 Bass kernel for nn_AttentiveEncoderPOS — v2: sequence-parallel.

Strategy (8 cores, SPMD, AllGather collective):
  - The linear (embed+concat+linear — the expensive replicated part of the
    baseline) is SHARDED: core i computes L rows [i*1024, (i+1)*1024) only
    (4.3 GF instead of 34.4 GF per core) directly in L.T layout (h on
    partitions), which doubles as Q.T for the attention.
  - L.T chunks are AllGathered in two row-halves (512 rows each) so the
    second half's collective overlaps the first half's attention math.
  - V (natural layout) is NOT gathered: each core transposes the gathered
    L.T tiles locally on the tensor engine (cheap) — saves a 16.8 MB
    collective + its HBM round trip.
  - Phase 2: each core attends its 1024 queries (Q.T resident in SBUF)
    against all 8192 keys, block by block. Softmax denominator accumulates
    on the vector engine (sum of exp tiles) and reduces with a ones-column
    matmul at the end.
  - Bias is folded into L (matches the reference exactly).
  - bf16 matmul inputs, fp32 PSUM accumulation. Scores are tiny (|s|<0.03)
    so exp() without max-subtraction is exact softmax.
"""

import numpy as np

import concourse.bass as bass
import concourse.mybir as mybir
from concourse import bacc
from concourse.tile import TileContext
from concourse.bass_utils import run_bass_kernel_spmd
from concourse.masks import make_identity

N = 8192
H = 1024
VOCAB = 50257
POS = 64
NCORES = 8
NL = N // NCORES          # 1024 rows per core
P = 128
RT = NL // P              # 8 row tiles per core
HT = H // P               # 8 h tiles
K2 = 2 * H
KTI = K2 // P             # 16 contraction tiles for the linear
HC = NL // 2              # 512-row half-chunks for the split AllGather
SCALE = 1.0 / 32.0        # 1/sqrt(H)

BF = mybir.dt.bfloat16
F32 = mybir.dt.float32
I32 = mybir.dt.int32
EXP = mybir.ActivationFunctionType.Exp


def build_nc():
    nc = bacc.Bacc(num_devices=NCORES)
    ids = nc.declare_dram_parameter("ids", [RT, P, 1], I32, isOutput=False)
    pids = nc.declare_dram_parameter("pids", [RT, P, 1], I32, isOutput=False)
    emb = nc.declare_dram_parameter("emb", [VOCAB, H], F32, isOutput=False)
    pemb = nc.declare_dram_parameter("pemb", [POS, H], F32, isOutput=False)
    wt = nc.declare_dram_parameter("wt", [K2, H], F32, isOutput=False)  # W.T
    bias = nc.declare_dram_parameter("bias", [HT, P, 1], F32, isOutput=False)
    out = nc.declare_dram_parameter("out", [NL, H], F32, isOutput=True)

    # AllGather buffers: L.T chunk in two row-halves
    lt_in = [nc.dram_tensor(f"lt_in{h}", [HT, P, HC], BF) for h in range(2)]
    lt_all = [
        nc.dram_tensor(f"lt_all{h}", [NCORES, HT, P, HC], BF, addr_space="Shared")
        for h in range(2)
    ]
    groups = [list(range(NCORES))]

    with TileContext(nc) as tc:
        with (
            tc.tile_pool(name="const", bufs=1) as const,
            tc.tile_pool(name="persist", bufs=1) as persist,
        ):
            ident = const.tile([P, P], BF)
            make_identity(nc, ident[:])
            ident32 = const.tile([P, P], F32)
            make_identity(nc, ident32[:])
            ones32 = const.tile([P, 1], F32)
            nc.gpsimd.memset(ones32[:], 1.0)
            b_sb = const.tile([P, HT], F32)
            nc.sync.dma_start(
                out=b_sb[:].rearrange("p (h u) -> p h u", h=HT),
                in_=bias.rearrange("h p u -> p h u"),
            )

            # persistent SBUF: own L.T (= Q.T), output acc, colsum acc
            qts = [persist.tile([P, NL], BF, name=f"qt{ht}") for ht in range(HT)]
            out_sb = [persist.tile([P, H], F32, name=f"o{qt}") for qt in range(RT)]
            cs_acc = persist.tile([P, NL], F32, name="cs_acc")
            nc.gpsimd.memset(cs_acc[:], 0.0)

            # ---------------- Phase 1: own L.T chunk ----------
            with (
                tc.tile_pool(name="wtp", bufs=KTI) as wtp,
                tc.tile_pool(name="wld", bufs=2) as wld,
                tc.tile_pool(name="idp", bufs=2 * RT) as idp,
                tc.tile_pool(name="xfp", bufs=3) as xfp,
                tc.tile_pool(name="xbp", bufs=RT) as xbp,
                tc.tile_pool(name="xtp", bufs=KTI) as xtp,
                tc.tile_pool(name="tps", bufs=3, space="PSUM") as tps,
                tc.tile_pool(name="mps", bufs=3, space="PSUM") as mps,
            ):
                # W.T -> bf16 SBUF
                wtb = []
                for k in range(KTI):
                    wf = wld.tile([P, H], F32, tag="wld")
                    nc.sync.dma_start(out=wf[:], in_=wt[k * P : (k + 1) * P, :])
                    wb = wtp.tile([P, H], BF, tag="wtb")
                    nc.vector.tensor_copy(out=wb[:], in_=wf[:])
                    wtb.append(wb)

                # gather own rows of X = concat(emb[ids], pemb[pids])
                xbs = []
                for rt in range(RT):
                    idt = idp.tile([P, 1], I32, tag="id")
                    nc.sync.dma_start(out=idt[:], in_=ids[rt])
                    pidt = idp.tile([P, 1], I32, tag="pid")
                    nc.sync.dma_start(out=pidt[:], in_=pids[rt])
                    xf = xfp.tile([P, K2], F32, tag="xf")
                    nc.gpsimd.indirect_dma_start(
                        out=xf[:, 0:H],
                        out_offset=None,
                        in_=emb[:],
                        in_offset=bass.IndirectOffsetOnAxis(ap=idt[:, :1], axis=0),
                    )
                    nc.gpsimd.indirect_dma_start(
                        out=xf[:, H:K2],
                        out_offset=None,
                        in_=pemb[:],
                        in_offset=bass.IndirectOffsetOnAxis(ap=pidt[:, :1], axis=0),
                    )
                    xb = xbp.tile([P, K2], BF, tag="xb")
                    nc.vector.tensor_copy(out=xb[:], in_=xf[:])
                    xbs.append(xb)

                # transpose X -> X.T tiles [128(2h), 1024 rows]
                xts = []
                for k in range(KTI):
                    pt = tps.tile([P, NL], BF, tag="tp")
                    for rt in range(RT):
                        nc.tensor.transpose(
                            pt[:, rt * P : (rt + 1) * P],
                            xbs[rt][:, k * P : (k + 1) * P],
                            ident[:],
                        )
                    xt = xtp.tile([P, NL], BF, tag="xt")
                    nc.vector.tensor_copy(out=xt[:], in_=pt[:])
                    xts.append(xt)

                # linear by row-half, AllGather each half as it completes
                for rc in range(2):
                    for ht in range(HT):
                        pm = mps.tile([P, HC], F32, tag="mp")
                        for k in range(KTI):
                            nc.tensor.matmul(
                                pm[:],
                                lhsT=wtb[k][:, ht * P : (ht + 1) * P],
                                rhs=xts[k][:, rc * HC : (rc + 1) * HC],
                                start=(k == 0),
                                stop=(k == KTI - 1),
                            )
                        nc.vector.tensor_add(
                            out=qts[ht][:, rc * HC : (rc + 1) * HC],
                            in0=pm[:],
                            in1=b_sb[:, ht : ht + 1].to_broadcast([P, HC]),
                        )
                        nc.sync.dma_start(
                            out=lt_in[rc][ht],
                            in_=qts[ht][:, rc * HC : (rc + 1) * HC],
                        )
                    nc.gpsimd.collective_compute(
                        "AllGather",
                        mybir.AluOpType.bypass,
                        replica_groups=groups,
                        ins=[lt_in[rc][:].opt()],
                        outs=[lt_all[rc][:].opt()],
                    )

            # ---------------- Phase 2: attention ----------------
            with (
                tc.tile_pool(name="lkp", bufs=2 * HT) as lkp,
                tc.tile_pool(name="vp2", bufs=2 * RT) as vp2,
                tc.tile_pool(name="ep", bufs=2 * RT) as ep,
                tc.tile_pool(name="fin", bufs=2) as fin,
            ):
              with (
                tc.tile_pool(name="sps", bufs=2, space="PSUM") as sps,
                tc.tile_pool(name="tp2", bufs=2, space="PSUM") as tp2,
                tc.tile_pool(name="ops", bufs=2, space="PSUM") as ops,
              ):
                for b in range(NCORES):
                    es = []
                    vts = []
                    for half in range(2):
                        # keys [b*1024 + half*512, +512): L.T tiles
                        ltc = []
                        for ht in range(HT):
                            t = lkp.tile([P, HC], BF, tag="lk")
                            nc.sync.dma_start(out=t[:], in_=lt_all[half][b, ht])
                            ltc.append(t)
                        # V tiles for these 512 keys via local transpose
                        for rt in range(HC // P):
                            ptv = tp2.tile([P, H], BF, tag="tpv")
                            for ht in range(HT):
                                nc.tensor.transpose(
                                    ptv[:, ht * P : (ht + 1) * P],
                                    ltc[ht][:, rt * P : (rt + 1) * P],
                                    ident[:],
                                )
                            vt = vp2.tile([P, H], BF, tag="v2")
                            nc.vector.tensor_copy(out=vt[:], in_=ptv[:])
                            vts.append(vt)
                        # scores + exp for each 128-key tile
                        for kt in range(HC // P):
                            e = ep.tile([P, NL], BF, tag="e")
                            for qc in range(NL // 512):
                                ps = sps.tile([P, 512], F32, tag="sp")
                                for ht in range(HT):
                                    nc.tensor.matmul(
                                        ps[:],
                                        lhsT=ltc[ht][:, kt * P : (kt + 1) * P],
                                        rhs=qts[ht][:, qc * 512 : (qc + 1) * 512],
                                        start=(ht == 0),
                                        stop=(ht == HT - 1),
                                    )
                                nc.scalar.activation(
                                    out=e[:, qc * 512 : (qc + 1) * 512],
                                    in_=ps[:],
                                    func=EXP,
                                    scale=SCALE,
                                )
                            # denominator partial sums on DVE
                            nc.vector.tensor_add(
                                out=cs_acc[:], in0=cs_acc[:], in1=e[:]
                            )
                            es.append(e)

                    # A @ V for this 1024-key block, accumulate into out_sb
                    for qt in range(RT):
                        po = ops.tile([P, H], F32, tag="op")
                        for j in range(RT):
                            for hh in range(H // 512):
                                nc.tensor.matmul(
                                    po[:, hh * 512 : (hh + 1) * 512],
                                    lhsT=es[j][:, qt * P : (qt + 1) * P],
                                    rhs=vts[j][:, hh * 512 : (hh + 1) * 512],
                                    start=(j == 0),
                                    stop=(j == RT - 1),
                                )
                        if b == 0:
                            nc.vector.tensor_copy(out=out_sb[qt][:], in_=po[:])
                        else:
                            nc.vector.tensor_add(
                                out=out_sb[qt][:], in0=out_sb[qt][:], in1=po[:]
                            )

              # denominator: reduce cs_acc over key-partitions (ones-column
              # fp32 matmul), then reciprocal + transpose to per-q layout
              with tc.tile_pool(name="fps", bufs=2, space="PSUM") as fps:
                cs_row = fin.tile([1, NL], F32, tag="cs_row")
                for qc in range(NL // 512):
                    cp = fps.tile([1, 512], F32, tag="cp")
                    nc.tensor.matmul(
                        cp[:],
                        lhsT=ones32[:],
                        rhs=cs_acc[:, qc * 512 : (qc + 1) * 512],
                        start=True,
                        stop=True,
                    )
                    nc.vector.reciprocal(
                        cs_row[0:1, qc * 512 : (qc + 1) * 512], cp[:]
                    )
                rec_ps = fps.tile([P, RT], F32, tag="rt")
                for qt in range(RT):
                    nc.tensor.transpose(
                        rec_ps[:, qt : qt + 1],
                        cs_row[0:1, qt * P : (qt + 1) * P],
                        ones32[0:1, 0:1],
                    )
                rec = fin.tile([P, RT], F32, tag="rec")
                nc.vector.tensor_copy(out=rec[:], in_=rec_ps[:])
                for qt in range(RT):
                    nc.vector.tensor_mul(
                        out=out_sb[qt][:],
                        in0=out_sb[qt][:],
                        in1=rec[:, qt : qt + 1].to_broadcast([P, H]),
                    )
                    nc.sync.dma_start(
                        out=out[qt * P : (qt + 1) * P, :], in_=out_sb[qt][:]
                    )
    nc.finalize()
    return nc


def _prep_inputs(inputs):
    ids = np.asarray(inputs["input_ids"]).astype(np.int32)
    pids = np.asarray(inputs["pos_ids"]).astype(np.int32)
    emb = np.asarray(inputs["emb"], dtype=np.float32)
    pemb = np.asarray(inputs["pos_emb"], dtype=np.float32)
    W = np.asarray(inputs["W"], dtype=np.float32)
    b = np.asarray(inputs["b"], dtype=np.float32)
    wt = np.ascontiguousarray(W.T)                      # [2H, H]
    bias = np.ascontiguousarray(b.reshape(HT, P, 1))
    in_maps = []
    for i in range(NCORES):
        r = ids[i * NL : (i + 1) * NL]
        rp = pids[i * NL : (i + 1) * NL]
        in_maps.append(
            {
                "ids": np.ascontiguousarray(r.reshape(RT, P, 1)),
                "pids": np.ascontiguousarray(rp.reshape(RT, P, 1)),
                "emb": emb,
                "pemb": pemb,
                "wt": wt,
                "bias": bias,
            }
        )
    return in_maps


def run(inputs, trace=False):
    nc = build_nc()
    in_maps = _prep_inputs(inputs)
    res = run_bass_kernel_spmd(nc, in_maps, list(range(NCORES)), trace=trace)
    out = np.concatenate([res.results[i]["out"] for i in range(NCORES)], axis=0)
    return out, res


def kernel(**inputs):
    out, _ = run(inputs, trace=False)
    return out


# revision 3
# speedup vs baseline: 1.6104x; 1.0800x over previous
"""Trainium2 Bass kernel for nn_AttentiveEncoderPOS — v3: fp8 DoubleRow attention.

Same sequence-parallel structure as v2 (sharded linear + split AllGather of
L.T + local V transposes), with phase 2 in fp8e4 using DoubleRow matmuls
(2 fp8 contraction elements per PE cell -> ~1.5-2x matmul throughput):
  - L.T is stored/gathered in fp8; Q.T, keys, V and exp(scores) all fp8.
  - DoubleRow pairs adjacent 128-wide contraction tiles: operands are laid
    out as [128, 2, free] access patterns over pair-tiles.
  - Scores contraction (h, 1024) -> 4 DoubleRow matmuls; A@V contraction
    (keys, per 1024-key block) -> 4 DoubleRow matmuls per output tile.
  - fp8 quantization error averages out over 8192 keys (verified on CPU:
    rel err ~1.3e-3 vs the 2e-2 gate).
The linear itself stays bf16 (X, W) with fp32 accumulation.
"""

import numpy as np

import concourse.bass as bass
import concourse.mybir as mybir
from concourse import bacc
from concourse.tile import TileContext
from concourse.bass_utils import run_bass_kernel_spmd
from concourse.masks import make_identity

N = 8192
H = 1024
VOCAB = 50257
POS = 64
NCORES = 8
NL = N // NCORES          # 1024 rows per core
P = 128
RT = NL // P              # 8 row tiles per core
HT = H // P               # 8 h tiles
GT = HT // 2              # 4 h-tile pairs (DoubleRow)
K2 = 2 * H
KTI = K2 // P             # 16 contraction tiles for the linear
HC = NL // 2              # 512-row half-chunks for the split AllGather
SCALE = 1.0 / 32.0        # 1/sqrt(H)

BF = mybir.dt.bfloat16
F32 = mybir.dt.float32
F8 = mybir.dt.float8e4
I32 = mybir.dt.int32
EXP = mybir.ActivationFunctionType.Exp
DR = mybir.MatmulPerfMode.DoubleRow


def pair(t, j2=2):
    """[128, 2*F] tile -> [128, 2, F] access pattern for DoubleRow."""
    return t[:].rearrange("p (j f) -> p j f", j=j2)


def build_nc():
    nc = bacc.Bacc(num_devices=NCORES)
    ids = nc.declare_dram_parameter("ids", [RT, P, 1], I32, isOutput=False)
    pids = nc.declare_dram_parameter("pids", [RT, P, 1], I32, isOutput=False)
    emb = nc.declare_dram_parameter("emb", [VOCAB, H], F32, isOutput=False)
    pemb = nc.declare_dram_parameter("pemb", [POS, H], F32, isOutput=False)
    wt = nc.declare_dram_parameter("wt", [K2, H], F32, isOutput=False)  # W.T
    bias = nc.declare_dram_parameter("bias", [HT, P, 1], F32, isOutput=False)
    out = nc.declare_dram_parameter("out", [NL, H], F32, isOutput=True)

    # AllGather buffers: L.T chunk (fp8) in two row-halves
    lt_in = [nc.dram_tensor(f"lt_in{h}", [HT, P, HC], F8) for h in range(2)]
    lt_all = [
        nc.dram_tensor(f"lt_all{h}", [NCORES, HT, P, HC], F8, addr_space="Shared")
        for h in range(2)
    ]
    groups = [list(range(NCORES))]

    with TileContext(nc) as tc:
        with (
            tc.tile_pool(name="const", bufs=1) as const,
            tc.tile_pool(name="persist", bufs=1) as persist,
        ):
            ident8 = const.tile([P, P], F8)
            make_identity(nc, ident8[:])
            ident32 = const.tile([P, P], F32)
            make_identity(nc, ident32[:])
            ident = const.tile([P, P], BF)
            make_identity(nc, ident[:])
            b_sb = const.tile([P, HT], F32)
            nc.sync.dma_start(
                out=b_sb[:].rearrange("p (h u) -> p h u", h=HT),
                in_=bias.rearrange("h p u -> p h u"),
            )

            # persistent SBUF: own Q.T in fp8 pair layout, outputs, colsum
            q8 = [persist.tile([P, 2 * NL], F8, name=f"q8{g}") for g in range(GT)]
            out_sb = [persist.tile([P, H], F32, name=f"o{qt}") for qt in range(RT)]
            cs_acc = persist.tile([P, NL], F32, name="cs_acc")
            nc.gpsimd.memset(cs_acc[:], 0.0)

            # ---------------- Phase 1: own L.T chunk (fp8) ----------
            with (
                tc.tile_pool(name="wtp", bufs=KTI) as wtp,
                tc.tile_pool(name="wld", bufs=2) as wld,
                tc.tile_pool(name="idp", bufs=2 * RT) as idp,
                tc.tile_pool(name="xfp", bufs=3) as xfp,
                tc.tile_pool(name="xbp", bufs=RT) as xbp,
                tc.tile_pool(name="xtp", bufs=KTI) as xtp,
                tc.tile_pool(name="tps", bufs=3, space="PSUM") as tps,
                tc.tile_pool(name="mps", bufs=3, space="PSUM") as mps,
            ):
                # gather own rows of X = concat(emb[ids], pemb[pids]) first
                xbs = []
                for rt in range(RT):
                    idt = idp.tile([P, 1], I32, tag="id")
                    nc.sync.dma_start(out=idt[:], in_=ids[rt])
                    pidt = idp.tile([P, 1], I32, tag="pid")
                    nc.sync.dma_start(out=pidt[:], in_=pids[rt])
                    xf = xfp.tile([P, K2], F32, tag="xf")
                    nc.gpsimd.indirect_dma_start(
                        out=xf[:, 0:H],
                        out_offset=None,
                        in_=emb[:],
                        in_offset=bass.IndirectOffsetOnAxis(ap=idt[:, :1], axis=0),
                    )
                    nc.gpsimd.indirect_dma_start(
                        out=xf[:, H:K2],
                        out_offset=None,
                        in_=pemb[:],
                        in_offset=bass.IndirectOffsetOnAxis(ap=pidt[:, :1], axis=0),
                    )
                    xb = xbp.tile([P, K2], BF, tag="xb")
                    nc.vector.tensor_copy(out=xb[:], in_=xf[:])
                    xbs.append(xb)

                # W.T -> bf16 SBUF (overlaps the gathers on the DMA side)
                wtb = []
                for k in range(KTI):
                    wf = wld.tile([P, H], F32, tag="wld")
                    nc.sync.dma_start(out=wf[:], in_=wt[k * P : (k + 1) * P, :])
                    wb = wtp.tile([P, H], BF, tag="wtb")
                    nc.vector.tensor_copy(out=wb[:], in_=wf[:])
                    wtb.append(wb)

                # transpose X -> X.T tiles [128(2h), 1024 rows]
                xts = []
                for k in range(KTI):
                    pt = tps.tile([P, NL], BF, tag="tp")
                    for rt in range(RT):
                        nc.tensor.transpose(
                            pt[:, rt * P : (rt + 1) * P],
                            xbs[rt][:, k * P : (k + 1) * P],
                            ident[:],
                        )
                    xt = xtp.tile([P, NL], BF, tag="xt")
                    nc.vector.tensor_copy(out=xt[:], in_=pt[:])
                    xts.append(xt)

                # linear by row-half; L.T goes straight to fp8 pair layout
                for rc in range(2):
                    for ht in range(HT):
                        pm = mps.tile([P, HC], F32, tag="mp")
                        for k in range(KTI):
                            nc.tensor.matmul(
                                pm[:],
                                lhsT=wtb[k][:, ht * P : (ht + 1) * P],
                                rhs=xts[k][:, rc * HC : (rc + 1) * HC],
                                start=(k == 0),
                                stop=(k == KTI - 1),
                            )
                        g, j = ht // 2, ht % 2
                        nc.vector.tensor_add(
                            out=q8[g][:, j * NL + rc * HC : j * NL + (rc + 1) * HC],
                            in0=pm[:],
                            in1=b_sb[:, ht : ht + 1].to_broadcast([P, HC]),
                        )
                        nc.sync.dma_start(
                            out=lt_in[rc][ht],
                            in_=q8[g][:, j * NL + rc * HC : j * NL + (rc + 1) * HC],
                        )
                    nc.gpsimd.collective_compute(
                        "AllGather",
                        mybir.AluOpType.bypass,
                        replica_groups=groups,
                        ins=[lt_in[rc][:].opt()],
                        outs=[lt_all[rc][:].opt()],
                    )

            # ---------------- Phase 2: attention (fp8 DoubleRow) ----------
            with (
                tc.tile_pool(name="lkp", bufs=4 * GT) as lkp,
                tc.tile_pool(name="vp2", bufs=2 * GT) as vp2,
                tc.tile_pool(name="ep", bufs=2 * GT) as ep,
                tc.tile_pool(name="fin", bufs=2) as fin,
            ):
              with (
                tc.tile_pool(name="sps", bufs=2, space="PSUM") as sps,
                tc.tile_pool(name="tp2", bufs=2, space="PSUM") as tp2,
                tc.tile_pool(name="ops", bufs=2, space="PSUM") as ops,
              ):
                for b in range(NCORES):
                    es = []
                    vts = []
                    for half in range(2):
                        # key L.T pair-tiles for these 512 keys
                        ltc = []
                        for g in range(GT):
                            t = lkp.tile([P, 2 * HC], F8, tag="lk")
                            nc.sync.dma_start(
                                out=t[:, 0:HC], in_=lt_all[half][b, 2 * g]
                            )
                            nc.sync.dma_start(
                                out=t[:, HC : 2 * HC], in_=lt_all[half][b, 2 * g + 1]
                            )
                            ltc.append(t)
                        # V pair-tiles via local transpose (fp8 through PE)
                        for ktp in range(2):  # two key-tile pairs per half
                            vt = vp2.tile([P, 2 * H], F8, tag="v2")
                            for j in range(2):
                                kt = ktp * 2 + j
                                # fp8 PE transpose requires output element
                                # step 2: write strided into a 2x-wide tile
                                ptv = tp2.tile([P, 2 * H], F8, tag="tpv")
                                pst = ptv[:].rearrange(
                                    "p (c two) -> p c two", two=2
                                )
                                for ht in range(HT):
                                    g, jj = ht // 2, ht % 2
                                    nc.tensor.transpose(
                                        pst[:, ht * P : (ht + 1) * P, 0:1],
                                        ltc[g][:, jj * HC + kt * P : jj * HC + (kt + 1) * P],
                                        ident8[:],
                                    )
                                nc.vector.tensor_copy(
                                    out=vt[:, j * H : (j + 1) * H],
                                    in_=pst[:, :, 0:1],
                                )
                            vts.append(vt)
                        # scores + exp, two key tiles share an e pair-tile
                        for ktp in range(2):
                            e = ep.tile([P, 2 * NL], F8, tag="e")
                            for j in range(2):
                                kt = ktp * 2 + j
                                for qc in range(NL // 512):
                                    ps = sps.tile([P, 512], F32, tag="sp")
                                    for g in range(GT):
                                        nc.tensor.matmul(
                                            ps[:],
                                            lhsT=pair(ltc[g])[
                                                :, :, kt * P : (kt + 1) * P
                                            ],
                                            rhs=pair(q8[g])[
                                                :, :, qc * 512 : (qc + 1) * 512
                                            ],
                                            start=(g == 0),
                                            stop=(g == GT - 1),
                                            perf_mode=DR,
                                        )
                                    nc.scalar.activation(
                                        out=e[:, j * NL + qc * 512 : j * NL + (qc + 1) * 512],
                                        in_=ps[:],
                                        func=EXP,
                                        scale=SCALE,
                                    )
                                nc.vector.tensor_add(
                                    out=cs_acc[:],
                                    in0=cs_acc[:],
                                    in1=e[:, j * NL : (j + 1) * NL],
                                )
                            es.append(e)

                    # last block: reduce the softmax denominator while AV runs
                    if b == NCORES - 1:
                        den = fin.tile([P, RT], F32, tag="den")
                        for qt in range(RT):
                            ct = sps.tile([P, P], F32, tag="sp")
                            nc.tensor.transpose(
                                ct[:],
                                cs_acc[:, qt * P : (qt + 1) * P],
                                ident32[:],
                            )
                            nc.vector.tensor_reduce(
                                out=den[:, qt : qt + 1],
                                in_=ct[:],
                                axis=mybir.AxisListType.X,
                                op=mybir.AluOpType.add,
                            )
                        rec = fin.tile([P, RT], F32, tag="rec")
                        nc.vector.reciprocal(rec[:], den[:])

                    # A @ V for this 1024-key block (4 DoubleRow pairs)
                    for qt in range(RT):
                        po = ops.tile([P, H], F32, tag="op")
                        for m in range(4):
                            for hh in range(H // 512):
                                nc.tensor.matmul(
                                    po[:, hh * 512 : (hh + 1) * 512],
                                    lhsT=pair(es[m])[:, :, qt * P : (qt + 1) * P],
                                    rhs=pair(vts[m])[:, :, hh * 512 : (hh + 1) * 512],
                                    start=(m == 0),
                                    stop=(m == 3),
                                    perf_mode=DR,
                                )
                        if b == 0:
                            nc.vector.tensor_copy(out=out_sb[qt][:], in_=po[:])
                        else:
                            nc.vector.tensor_add(
                                out=out_sb[qt][:], in0=out_sb[qt][:], in1=po[:]
                            )

                # normalize and write out
                for qt in range(RT):
                    nc.vector.tensor_mul(
                        out=out_sb[qt][:],
                        in0=out_sb[qt][:],
                        in1=rec[:, qt : qt + 1].to_broadcast([P, H]),
                    )
                    nc.sync.dma_start(
                        out=out[qt * P : (qt + 1) * P, :], in_=out_sb[qt][:]
                    )
    nc.finalize()
    return nc


def _prep_inputs(inputs):
    ids = np.asarray(inputs["input_ids"]).astype(np.int32)
    pids = np.asarray(inputs["pos_ids"]).astype(np.int32)
    emb = np.asarray(inputs["emb"], dtype=np.float32)
    pemb = np.asarray(inputs["pos_emb"], dtype=np.float32)
    W = np.asarray(inputs["W"], dtype=np.float32)
    b = np.asarray(inputs["b"], dtype=np.float32)
    wt = np.ascontiguousarray(W.T)                      # [2H, H]
    bias = np.ascontiguousarray(b.reshape(HT, P, 1))
    in_maps = []
    for i in range(NCORES):
        r = ids[i * NL : (i + 1) * NL]
        rp = pids[i * NL : (i + 1) * NL]
        in_maps.append(
            {
                "ids": np.ascontiguousarray(r.reshape(RT, P, 1)),
                "pids": np.ascontiguousarray(rp.reshape(RT, P, 1)),
                "emb": emb,
                "pemb": pemb,
                "wt": wt,
                "bias": bias,
            }
        )
    return in_maps


def run(inputs, trace=False):
    nc = build_nc()
    in_maps = _prep_inputs(inputs)
    res = run_bass_kernel_spmd(nc, in_maps, list(range(NCORES)), trace=trace)
    out = np.concatenate([res.results[i]["out"] for i in range(NCORES)], axis=0)
    return out, res


def kernel(**inputs):
    out, _ = run(inputs, trace=False)
    return out


# revision 4
# speedup vs baseline: 1.6635x; 1.0330x over previous
"""Trainium2 Bass kernel for nn_AttentiveEncoderPOS — v4.

v3 (fp8 DoubleRow attention) plus:
  - Phase 1 pipelined by row-half: gather/transpose/linear/AllGather per
    512-row half so the first collective starts ~60us earlier.
  - Phase 2 processes blocks in two groups of 4; A@V accumulates across a
    whole group in PSUM (16-matmul chains), cutting output-accumulate DVE
    traffic 4x.
  - V-tile copies (PSUM->SBUF) moved to the scalar engine, softmax
    denominator accumulation moved to gpsimd — the vector engine was
    becoming a co-bottleneck in v3.
"""

import numpy as np

import concourse.bass as bass
import concourse.mybir as mybir
from concourse import bacc
from concourse.tile import TileContext
from concourse.bass_utils import run_bass_kernel_spmd
from concourse.masks import make_identity

N = 8192
H = 1024
VOCAB = 50257
POS = 64
NCORES = 8
NL = N // NCORES          # 1024 rows per core
P = 128
RT = NL // P              # 8 row tiles per core
HT = H // P               # 8 h tiles
GT = HT // 2              # 4 h-tile pairs (DoubleRow)
K2 = 2 * H
KTI = K2 // P             # 16 contraction tiles for the linear
HC = NL // 2              # 512-row half-chunks for the split AllGather
GRP = 4                   # blocks per A@V accumulation group
SCALE = 1.0 / 32.0        # 1/sqrt(H)

BF = mybir.dt.bfloat16
F32 = mybir.dt.float32
F8 = mybir.dt.float8e4
I32 = mybir.dt.int32
EXP = mybir.ActivationFunctionType.Exp
DR = mybir.MatmulPerfMode.DoubleRow


def pair(t, j2=2):
    """[128, 2*F] tile -> [128, 2, F] access pattern for DoubleRow."""
    return t[:].rearrange("p (j f) -> p j f", j=j2)


def build_nc():
    nc = bacc.Bacc(num_devices=NCORES)
    ids = nc.declare_dram_parameter("ids", [RT, P, 1], I32, isOutput=False)
    pids = nc.declare_dram_parameter("pids", [RT, P, 1], I32, isOutput=False)
    emb = nc.declare_dram_parameter("emb", [VOCAB, H], F32, isOutput=False)
    pemb = nc.declare_dram_parameter("pemb", [POS, H], F32, isOutput=False)
    wt = nc.declare_dram_parameter("wt", [K2, H], F32, isOutput=False)  # W.T
    bias = nc.declare_dram_parameter("bias", [HT, P, 1], F32, isOutput=False)
    out = nc.declare_dram_parameter("out", [NL, H], F32, isOutput=True)

    # AllGather buffers: L.T chunk (fp8) in two row-halves
    lt_in = [nc.dram_tensor(f"lt_in{h}", [HT, P, HC], F8) for h in range(2)]
    lt_all = [
        nc.dram_tensor(f"lt_all{h}", [NCORES, HT, P, HC], F8, addr_space="Shared")
        for h in range(2)
    ]
    groups = [list(range(NCORES))]

    with TileContext(nc) as tc:
        with (
            tc.tile_pool(name="const", bufs=1) as const,
            tc.tile_pool(name="persist", bufs=1) as persist,
        ):
            ident8 = const.tile([P, P], F8)
            make_identity(nc, ident8[:])
            ident32 = const.tile([P, P], F32)
            make_identity(nc, ident32[:])
            ident = const.tile([P, P], BF)
            make_identity(nc, ident[:])
            b_sb = const.tile([P, HT], F32)
            nc.sync.dma_start(
                out=b_sb[:].rearrange("p (h u) -> p h u", h=HT),
                in_=bias.rearrange("h p u -> p h u"),
            )

            # persistent SBUF: own Q.T in fp8 pair layout, outputs, colsum
            q8 = [persist.tile([P, 2 * NL], F8, name=f"q8{g}") for g in range(GT)]
            out_sb = [persist.tile([P, H], F32, name=f"o{qt}") for qt in range(RT)]
            cs_acc = persist.tile([P, NL], F32, name="cs_acc")
            nc.gpsimd.memset(cs_acc[:], 0.0)

            # ---------------- Phase 1: own L.T chunk (fp8), half-pipelined --
            with (
                tc.tile_pool(name="wtp", bufs=KTI) as wtp,
                tc.tile_pool(name="wld", bufs=2) as wld,
                tc.tile_pool(name="idp", bufs=2 * RT) as idp,
                tc.tile_pool(name="xfp", bufs=3) as xfp,
                tc.tile_pool(name="xbp", bufs=RT) as xbp,
                tc.tile_pool(name="xtp", bufs=2 * KTI) as xtp,
                tc.tile_pool(name="tps", bufs=3, space="PSUM") as tps,
                tc.tile_pool(name="mps", bufs=3, space="PSUM") as mps,
            ):
                idts, pidts = [], []
                for rt in range(RT):
                    idt = idp.tile([P, 1], I32, tag="id")
                    nc.sync.dma_start(out=idt[:], in_=ids[rt])
                    idts.append(idt)
                    pidt = idp.tile([P, 1], I32, tag="pid")
                    nc.sync.dma_start(out=pidt[:], in_=pids[rt])
                    pidts.append(pidt)

                wtb = []
                for rc in range(2):
                    # gather this half's rows of X = concat(emb[.], pemb[.])
                    xbs = []
                    for rt in range(rc * 4, rc * 4 + 4):
                        xf = xfp.tile([P, K2], F32, tag="xf")
                        nc.gpsimd.indirect_dma_start(
                            out=xf[:, 0:H],
                            out_offset=None,
                            in_=emb[:],
                            in_offset=bass.IndirectOffsetOnAxis(
                                ap=idts[rt][:, :1], axis=0
                            ),
                        )
                        nc.gpsimd.indirect_dma_start(
                            out=xf[:, H:K2],
                            out_offset=None,
                            in_=pemb[:],
                            in_offset=bass.IndirectOffsetOnAxis(
                                ap=pidts[rt][:, :1], axis=0
                            ),
                        )
                        xb = xbp.tile([P, K2], BF, tag="xb")
                        nc.vector.tensor_copy(out=xb[:], in_=xf[:])
                        xbs.append(xb)

                    if rc == 0:
                        # W.T loads ride the sync DMA queue behind the id
                        # loads while the gathers run on gpsimd
                        for k in range(KTI):
                            wf = wld.tile([P, H], F32, tag="wld")
                            nc.sync.dma_start(
                                out=wf[:], in_=wt[k * P : (k + 1) * P, :]
                            )
                            wb = wtp.tile([P, H], BF, tag="wtb")
                            nc.vector.tensor_copy(out=wb[:], in_=wf[:])
                            wtb.append(wb)

                    # transpose this half of X -> X.T tiles [128(2h), 512]
                    xth = []
                    for k in range(KTI):
                        pt = tps.tile([P, HC], BF, tag="tp")
                        for i, xb in enumerate(xbs):
                            nc.tensor.transpose(
                                pt[:, i * P : (i + 1) * P],
                                xb[:, k * P : (k + 1) * P],
                                ident[:],
                            )
                        xt = xtp.tile([P, HC], BF, tag="xt")
                        nc.vector.tensor_copy(out=xt[:], in_=pt[:])
                        xth.append(xt)

                    # linear for this half; L.T goes to fp8 pair layout
                    for ht in range(HT):
                        pm = mps.tile([P, HC], F32, tag="mp")
                        for k in range(KTI):
                            nc.tensor.matmul(
                                pm[:],
                                lhsT=wtb[k][:, ht * P : (ht + 1) * P],
                                rhs=xth[k][:],
                                start=(k == 0),
                                stop=(k == KTI - 1),
                            )
                        g, j = ht // 2, ht % 2
                        nc.vector.tensor_add(
                            out=q8[g][:, j * NL + rc * HC : j * NL + (rc + 1) * HC],
                            in0=pm[:],
                            in1=b_sb[:, ht : ht + 1].to_broadcast([P, HC]),
                        )
                        nc.sync.dma_start(
                            out=lt_in[rc][ht],
                            in_=q8[g][:, j * NL + rc * HC : j * NL + (rc + 1) * HC],
                        )
                    nc.gpsimd.collective_compute(
                        "AllGather",
                        mybir.AluOpType.bypass,
                        replica_groups=groups,
                        ins=[lt_in[rc][:].opt()],
                        outs=[lt_all[rc][:].opt()],
                    )

            # ---------------- Phase 2: attention (fp8 DoubleRow) ----------
            with (
                tc.tile_pool(name="lkp", bufs=4 * GT) as lkp,
                tc.tile_pool(name="vp2", bufs=(GRP + 1) * 4) as vp2,
                tc.tile_pool(name="ep", bufs=(GRP + 1) * 4) as ep,
                tc.tile_pool(name="fin", bufs=2) as fin,
            ):
              with (
                tc.tile_pool(name="sps", bufs=2, space="PSUM") as sps,
                tc.tile_pool(name="tp2", bufs=2, space="PSUM") as tp2,
                tc.tile_pool(name="ops", bufs=2, space="PSUM") as ops,
              ):
                for grp in range(NCORES // GRP):
                    es = []
                    vts = []
                    for bb in range(GRP):
                        b = grp * GRP + bb
                        for half in range(2):
                            # key L.T pair-tiles for these 512 keys
                            ltc = []
                            for g in range(GT):
                                t = lkp.tile([P, 2 * HC], F8, tag="lk")
                                nc.sync.dma_start(
                                    out=t[:, 0:HC], in_=lt_all[half][b, 2 * g]
                                )
                                nc.sync.dma_start(
                                    out=t[:, HC : 2 * HC],
                                    in_=lt_all[half][b, 2 * g + 1],
                                )
                                ltc.append(t)
                            # V pair-tiles via local fp8 transpose (stride-2
                            # PSUM out), drained by the scalar engine
                            for ktp in range(2):
                                vt = vp2.tile([P, 2 * H], F8, tag="v2")
                                for j in range(2):
                                    kt = ktp * 2 + j
                                    ptv = tp2.tile([P, 2 * H], F8, tag="tpv")
                                    pst = ptv[:].rearrange(
                                        "p (c two) -> p c two", two=2
                                    )
                                    for ht in range(HT):
                                        g, jj = ht // 2, ht % 2
                                        nc.tensor.transpose(
                                            pst[:, ht * P : (ht + 1) * P, 0:1],
                                            ltc[g][:, jj * HC + kt * P : jj * HC + (kt + 1) * P],
                                            ident8[:],
                                        )
                                    nc.scalar.copy(
                                        out=vt[:, j * H : (j + 1) * H],
                                        in_=pst[:, :, 0:1],
                                    )
                                vts.append(vt)
                            # scores + exp; denominator partials on gpsimd
                            for ktp in range(2):
                                e = ep.tile([P, 2 * NL], F8, tag="e")
                                for j in range(2):
                                    kt = ktp * 2 + j
                                    for qc in range(NL // 512):
                                        ps = sps.tile([P, 512], F32, tag="sp")
                                        for g in range(GT):
                                            nc.tensor.matmul(
                                                ps[:],
                                                lhsT=pair(ltc[g])[
                                                    :, :, kt * P : (kt + 1) * P
                                                ],
                                                rhs=pair(q8[g])[
                                                    :, :, qc * 512 : (qc + 1) * 512
                                                ],
                                                start=(g == 0),
                                                stop=(g == GT - 1),
                                                perf_mode=DR,
                                            )
                                        nc.scalar.activation(
                                            out=e[:, j * NL + qc * 512 : j * NL + (qc + 1) * 512],
                                            in_=ps[:],
                                            func=EXP,
                                            scale=SCALE,
                                        )
                                    nc.gpsimd.tensor_add(
                                        out=cs_acc[:],
                                        in0=cs_acc[:],
                                        in1=e[:, j * NL : (j + 1) * NL],
                                    )
                                es.append(e)

                        # last block overall: reduce denominator while the
                        # remaining matmuls run
                        if b == NCORES - 1:
                            den = fin.tile([P, RT], F32, tag="den")
                            for qt in range(RT):
                                ct = sps.tile([P, P], F32, tag="sp")
                                nc.tensor.transpose(
                                    ct[:],
                                    cs_acc[:, qt * P : (qt + 1) * P],
                                    ident32[:],
                                )
                                nc.vector.tensor_reduce(
                                    out=den[:, qt : qt + 1],
                                    in_=ct[:],
                                    axis=mybir.AxisListType.X,
                                    op=mybir.AluOpType.add,
                                )
                            rec = fin.tile([P, RT], F32, tag="rec")
                            nc.vector.reciprocal(rec[:], den[:])

                    # A @ V for the whole 4096-key group: 16-matmul chains
                    for qt in range(RT):
                        po = ops.tile([P, H], F32, tag="op")
                        for m in range(4 * GRP):
                            for hh in range(H // 512):
                                nc.tensor.matmul(
                                    po[:, hh * 512 : (hh + 1) * 512],
                                    lhsT=pair(es[m])[:, :, qt * P : (qt + 1) * P],
                                    rhs=pair(vts[m])[:, :, hh * 512 : (hh + 1) * 512],
                                    start=(m == 0),
                                    stop=(m == 4 * GRP - 1),
                                    perf_mode=DR,
                                )
                        if grp == 0:
                            nc.vector.tensor_copy(out=out_sb[qt][:], in_=po[:])
                        else:
                            nc.vector.tensor_add(
                                out=out_sb[qt][:], in0=out_sb[qt][:], in1=po[:]
                            )

                # normalize and write out
                for qt in range(RT):
                    nc.vector.tensor_mul(
                        out=out_sb[qt][:],
                        in0=out_sb[qt][:],
                        in1=rec[:, qt : qt + 1].to_broadcast([P, H]),
                    )
                    nc.sync.dma_start(
                        out=out[qt * P : (qt + 1) * P, :], in_=out_sb[qt][:]
                    )
    nc.finalize()
    return nc


def _prep_inputs(inputs):
    ids = np.asarray(inputs["input_ids"]).astype(np.int32)
    pids = np.asarray(inputs["pos_ids"]).astype(np.int32)
    emb = np.asarray(inputs["emb"], dtype=np.float32)
    pemb = np.asarray(inputs["pos_emb"], dtype=np.float32)
    W = np.asarray(inputs["W"], dtype=np.float32)
    b = np.asarray(inputs["b"], dtype=np.float32)
    wt = np.ascontiguousarray(W.T)                      # [2H, H]
    bias = np.ascontiguousarray(b.reshape(HT, P, 1))
    in_maps = []
    for i in range(NCORES):
        r = ids[i * NL : (i + 1) * NL]
        rp = pids[i * NL : (i + 1) * NL]
        in_maps.append(
            {
                "ids": np.ascontiguousarray(r.reshape(RT, P, 1)),
                "pids": np.ascontiguousarray(rp.reshape(RT, P, 1)),
                "emb": emb,
                "pemb": pemb,
                "wt": wt,
                "bias": bias,
            }
        )
    return in_maps


def run(inputs, trace=False):
    nc = build_nc()
    in_maps = _prep_inputs(inputs)
    res = run_bass_kernel_spmd(nc, in_maps, list(range(NCORES)), trace=trace)
    out = np.concatenate([res.results[i]["out"] for i in range(NCORES)], axis=0)
    return out, res


def kernel(**inputs):
    out, _ = run(inputs, trace=False)
    return out


# revision 5
# speedup vs baseline: 1.7001x; 1.0220x over previous
"""Trainium2 Bass kernel for nn_AttentiveEncoderPOS — v5.

v4 plus:
  - V is AllGathered in fp8 (third collective, fully hidden behind scores)
    instead of being re-transposed per block on every core: each core
    transposes only its OWN 1024 rows once in phase 1. Removes 448 PE
    transposes and all strided scalar-engine drain copies from phase 2.
  - Group scores are ordered half-0-of-all-blocks first so the second lt
    AllGather is hidden behind half-0 compute.
"""

import numpy as np

import concourse.bass as bass
import concourse.mybir as mybir
from concourse import bacc
from concourse.tile import TileContext
from concourse.bass_utils import run_bass_kernel_spmd
from concourse.masks import make_identity

N = 8192
H = 1024
VOCAB = 50257
POS = 64
NCORES = 8
NL = N // NCORES          # 1024 rows per core
P = 128
RT = NL // P              # 8 row tiles per core
HT = H // P               # 8 h tiles
GT = HT // 2              # 4 h-tile pairs (DoubleRow)
K2 = 2 * H
KTI = K2 // P             # 16 contraction tiles for the linear
HC = NL // 2              # 512-row half-chunks for the split AllGather
GRP = 4                   # blocks per A@V accumulation group
SCALE = 1.0 / 32.0        # 1/sqrt(H)

BF = mybir.dt.bfloat16
F32 = mybir.dt.float32
F8 = mybir.dt.float8e4
I32 = mybir.dt.int32
EXP = mybir.ActivationFunctionType.Exp
DR = mybir.MatmulPerfMode.DoubleRow


def pair(t, j2=2):
    """[128, 2*F] tile -> [128, 2, F] access pattern for DoubleRow."""
    return t[:].rearrange("p (j f) -> p j f", j=j2)


def build_nc():
    nc = bacc.Bacc(num_devices=NCORES)
    ids = nc.declare_dram_parameter("ids", [RT, P, 1], I32, isOutput=False)
    pids = nc.declare_dram_parameter("pids", [RT, P, 1], I32, isOutput=False)
    emb = nc.declare_dram_parameter("emb", [VOCAB, H], F32, isOutput=False)
    pemb = nc.declare_dram_parameter("pemb", [POS, H], F32, isOutput=False)
    wt = nc.declare_dram_parameter("wt", [K2, H], F32, isOutput=False)  # W.T
    bias = nc.declare_dram_parameter("bias", [HT, P, 1], F32, isOutput=False)
    out = nc.declare_dram_parameter("out", [NL, H], F32, isOutput=True)

    # AllGather buffers: L.T chunk (fp8) in two row-halves, V chunk (fp8)
    lt_in = [nc.dram_tensor(f"lt_in{h}", [HT, P, HC], F8) for h in range(2)]
    lt_all = [
        nc.dram_tensor(f"lt_all{h}", [NCORES, HT, P, HC], F8, addr_space="Shared")
        for h in range(2)
    ]
    v_in = nc.dram_tensor("v_in", [RT, P, H], F8)
    v_all = nc.dram_tensor("v_all", [NCORES, RT, P, H], F8, addr_space="Shared")
    groups = [list(range(NCORES))]

    with TileContext(nc) as tc:
        with (
            tc.tile_pool(name="const", bufs=1) as const,
            tc.tile_pool(name="persist", bufs=1) as persist,
        ):
            ident8 = const.tile([P, P], F8)
            make_identity(nc, ident8[:])
            ident32 = const.tile([P, P], F32)
            make_identity(nc, ident32[:])
            ident = const.tile([P, P], BF)
            make_identity(nc, ident[:])
            b_sb = const.tile([P, HT], F32)
            nc.sync.dma_start(
                out=b_sb[:].rearrange("p (h u) -> p h u", h=HT),
                in_=bias.rearrange("h p u -> p h u"),
            )

            # persistent SBUF: own Q.T in fp8 pair layout, outputs, colsum
            q8 = [persist.tile([P, 2 * NL], F8, name=f"q8{g}") for g in range(GT)]
            out_sb = [persist.tile([P, H], F32, name=f"o{qt}") for qt in range(RT)]
            cs_acc = persist.tile([P, NL], F32, name="cs_acc")
            nc.gpsimd.memset(cs_acc[:], 0.0)

            # ---------------- Phase 1: own L.T chunk (fp8), half-pipelined --
            with (
                tc.tile_pool(name="wtp", bufs=KTI) as wtp,
                tc.tile_pool(name="wld", bufs=2) as wld,
                tc.tile_pool(name="idp", bufs=2 * RT) as idp,
                tc.tile_pool(name="xfp", bufs=3) as xfp,
                tc.tile_pool(name="xbp", bufs=RT) as xbp,
                tc.tile_pool(name="xtp", bufs=2 * KTI) as xtp,
                tc.tile_pool(name="tps", bufs=3, space="PSUM") as tps,
                tc.tile_pool(name="mps", bufs=3, space="PSUM") as mps,
            ):
                idts, pidts = [], []
                for rt in range(RT):
                    idt = idp.tile([P, 1], I32, tag="id")
                    nc.sync.dma_start(out=idt[:], in_=ids[rt])
                    idts.append(idt)
                    pidt = idp.tile([P, 1], I32, tag="pid")
                    nc.sync.dma_start(out=pidt[:], in_=pids[rt])
                    pidts.append(pidt)

                wtb = []
                for rc in range(2):
                    # gather this half's rows of X = concat(emb[.], pemb[.])
                    xbs = []
                    for rt in range(rc * 4, rc * 4 + 4):
                        xf = xfp.tile([P, K2], F32, tag="xf")
                        nc.gpsimd.indirect_dma_start(
                            out=xf[:, 0:H],
                            out_offset=None,
                            in_=emb[:],
                            in_offset=bass.IndirectOffsetOnAxis(
                                ap=idts[rt][:, :1], axis=0
                            ),
                        )
                        nc.gpsimd.indirect_dma_start(
                            out=xf[:, H:K2],
                            out_offset=None,
                            in_=pemb[:],
                            in_offset=bass.IndirectOffsetOnAxis(
                                ap=pidts[rt][:, :1], axis=0
                            ),
                        )
                        xb = xbp.tile([P, K2], BF, tag="xb")
                        nc.vector.tensor_copy(out=xb[:], in_=xf[:])
                        xbs.append(xb)

                    if rc == 0:
                        # W.T loads ride the sync DMA queue behind the id
                        # loads while the gathers run on gpsimd
                        for k in range(KTI):
                            wf = wld.tile([P, H], F32, tag="wld")
                            nc.sync.dma_start(
                                out=wf[:], in_=wt[k * P : (k + 1) * P, :]
                            )
                            wb = wtp.tile([P, H], BF, tag="wtb")
                            nc.vector.tensor_copy(out=wb[:], in_=wf[:])
                            wtb.append(wb)

                    # transpose this half of X -> X.T tiles [128(2h), 512]
                    xth = []
                    for k in range(KTI):
                        pt = tps.tile([P, HC], BF, tag="tp")
                        for i, xb in enumerate(xbs):
                            nc.tensor.transpose(
                                pt[:, i * P : (i + 1) * P],
                                xb[:, k * P : (k + 1) * P],
                                ident[:],
                            )
                        xt = xtp.tile([P, HC], BF, tag="xt")
                        nc.vector.tensor_copy(out=xt[:], in_=pt[:])
                        xth.append(xt)

                    # linear for this half; L.T goes to fp8 pair layout
                    for ht in range(HT):
                        pm = mps.tile([P, HC], F32, tag="mp")
                        for k in range(KTI):
                            nc.tensor.matmul(
                                pm[:],
                                lhsT=wtb[k][:, ht * P : (ht + 1) * P],
                                rhs=xth[k][:],
                                start=(k == 0),
                                stop=(k == KTI - 1),
                            )
                        g, j = ht // 2, ht % 2
                        nc.vector.tensor_add(
                            out=q8[g][:, j * NL + rc * HC : j * NL + (rc + 1) * HC],
                            in0=pm[:],
                            in1=b_sb[:, ht : ht + 1].to_broadcast([P, HC]),
                        )
                        nc.sync.dma_start(
                            out=lt_in[rc][ht],
                            in_=q8[g][:, j * NL + rc * HC : j * NL + (rc + 1) * HC],
                        )
                    nc.gpsimd.collective_compute(
                        "AllGather",
                        mybir.AluOpType.bypass,
                        replica_groups=groups,
                        ins=[lt_in[rc][:].opt()],
                        outs=[lt_all[rc][:].opt()],
                    )

                # own V tiles: transpose own L.T once, gather V as a third
                # collective (hidden behind the first score blocks)
                for rt in range(RT):
                    ptv = tps.tile([P, 2 * H], F8, tag="vt", bufs=2)
                    pst = ptv[:].rearrange("p (c two) -> p c two", two=2)
                    for ht in range(HT):
                        g, jj = ht // 2, ht % 2
                        nc.tensor.transpose(
                            pst[:, ht * P : (ht + 1) * P, 0:1],
                            q8[g][:, jj * NL + rt * P : jj * NL + (rt + 1) * P],
                            ident8[:],
                        )
                    vo = xbp.tile([P, H], F8, tag="vo")
                    nc.scalar.copy(out=vo[:], in_=pst[:, :, 0:1])
                    nc.sync.dma_start(out=v_in[rt], in_=vo[:])
                nc.gpsimd.collective_compute(
                    "AllGather",
                    mybir.AluOpType.bypass,
                    replica_groups=groups,
                    ins=[v_in[:].opt()],
                    outs=[v_all[:].opt()],
                )

            # ---------------- Phase 2: attention (fp8 DoubleRow) ----------
            with (
                tc.tile_pool(name="lkp", bufs=4 * GT) as lkp,
                tc.tile_pool(name="vp2", bufs=(GRP + 1) * 4) as vp2,
                tc.tile_pool(name="ep", bufs=(GRP + 1) * 4) as ep,
                tc.tile_pool(name="fin", bufs=2) as fin,
            ):
              with (
                tc.tile_pool(name="sps", bufs=4, space="PSUM") as sps,
                tc.tile_pool(name="ops", bufs=2, space="PSUM") as ops,
              ):
                for grp in range(NCORES // GRP):
                    es = [[None] * 4 for _ in range(GRP)]
                    vts = [[None] * 4 for _ in range(GRP)]
                    # half 0 of every block first: the second lt AllGather
                    # hides behind half-0 scores
                    for half in range(2):
                        for bb in range(GRP):
                            b = grp * GRP + bb
                            # key L.T pair-tiles for these 512 keys
                            ltc = []
                            for g in range(GT):
                                t = lkp.tile([P, 2 * HC], F8, tag="lk")
                                nc.sync.dma_start(
                                    out=t[:, 0:HC], in_=lt_all[half][b, 2 * g]
                                )
                                nc.sync.dma_start(
                                    out=t[:, HC : 2 * HC],
                                    in_=lt_all[half][b, 2 * g + 1],
                                )
                                ltc.append(t)
                            # V pair-tiles straight from the gathered buffer
                            for ktp in range(2):
                                m = half * 2 + ktp
                                vt = vp2.tile([P, 2 * H], F8, tag="v2")
                                nc.sync.dma_start(
                                    out=vt[:, 0:H], in_=v_all[b, 2 * m]
                                )
                                nc.sync.dma_start(
                                    out=vt[:, H : 2 * H], in_=v_all[b, 2 * m + 1]
                                )
                                vts[bb][m] = vt
                            # scores + exp; denominator partials on gpsimd
                            for ktp in range(2):
                                e = ep.tile([P, 2 * NL], F8, tag="e")
                                for j in range(2):
                                    kt = ktp * 2 + j
                                    for qc in range(NL // 512):
                                        ps = sps.tile([P, 512], F32, tag="sp")
                                        for g in range(GT):
                                            nc.tensor.matmul(
                                                ps[:],
                                                lhsT=pair(ltc[g])[
                                                    :, :, kt * P : (kt + 1) * P
                                                ],
                                                rhs=pair(q8[g])[
                                                    :, :, qc * 512 : (qc + 1) * 512
                                                ],
                                                start=(g == 0),
                                                stop=(g == GT - 1),
                                                perf_mode=DR,
                                            )
                                        nc.scalar.activation(
                                            out=e[:, j * NL + qc * 512 : j * NL + (qc + 1) * 512],
                                            in_=ps[:],
                                            func=EXP,
                                            scale=SCALE,
                                        )
                                    nc.gpsimd.tensor_add(
                                        out=cs_acc[:],
                                        in0=cs_acc[:],
                                        in1=e[:, j * NL : (j + 1) * NL],
                                    )
                                es[bb][half * 2 + ktp] = e

                    # after the last group's scores: reduce denominator while
                    # the remaining matmuls run
                    if grp == NCORES // GRP - 1:
                        den = fin.tile([P, RT], F32, tag="den")
                        for qt in range(RT):
                            ct = sps.tile([P, P], F32, tag="sp")
                            nc.tensor.transpose(
                                ct[:],
                                cs_acc[:, qt * P : (qt + 1) * P],
                                ident32[:],
                            )
                            nc.vector.tensor_reduce(
                                out=den[:, qt : qt + 1],
                                in_=ct[:],
                                axis=mybir.AxisListType.X,
                                op=mybir.AluOpType.add,
                            )
                        rec = fin.tile([P, RT], F32, tag="rec")
                        nc.vector.reciprocal(rec[:], den[:])

                    # A @ V for the whole 4096-key group: 32-matmul chains
                    for qt in range(RT):
                        po = ops.tile([P, H], F32, tag="op")
                        for mm in range(4 * GRP):
                            bb, m = mm // 4, mm % 4
                            for hh in range(H // 512):
                                nc.tensor.matmul(
                                    po[:, hh * 512 : (hh + 1) * 512],
                                    lhsT=pair(es[bb][m])[:, :, qt * P : (qt + 1) * P],
                                    rhs=pair(vts[bb][m])[:, :, hh * 512 : (hh + 1) * 512],
                                    start=(mm == 0),
                                    stop=(mm == 4 * GRP - 1),
                                    perf_mode=DR,
                                )
                        if grp == 0:
                            nc.vector.tensor_copy(out=out_sb[qt][:], in_=po[:])
                        else:
                            nc.vector.tensor_add(
                                out=out_sb[qt][:], in0=out_sb[qt][:], in1=po[:]
                            )

                # normalize and write out
                for qt in range(RT):
                    nc.vector.tensor_mul(
                        out=out_sb[qt][:],
                        in0=out_sb[qt][:],
                        in1=rec[:, qt : qt + 1].to_broadcast([P, H]),
                    )
                    nc.sync.dma_start(
                        out=out[qt * P : (qt + 1) * P, :], in_=out_sb[qt][:]
                    )
    nc.finalize()
    return nc


def _prep_inputs(inputs):
    ids = np.asarray(inputs["input_ids"]).astype(np.int32)
    pids = np.asarray(inputs["pos_ids"]).astype(np.int32)
    emb = np.asarray(inputs["emb"], dtype=np.float32)
    pemb = np.asarray(inputs["pos_emb"], dtype=np.float32)
    W = np.asarray(inputs["W"], dtype=np.float32)
    b = np.asarray(inputs["b"], dtype=np.float32)
    wt = np.ascontiguousarray(W.T)                      # [2H, H]
    bias = np.ascontiguousarray(b.reshape(HT, P, 1))
    in_maps = []
    for i in range(NCORES):
        r = ids[i * NL : (i + 1) * NL]
        rp = pids[i * NL : (i + 1) * NL]
        in_maps.append(
            {
                "ids": np.ascontiguousarray(r.reshape(RT, P, 1)),
                "pids": np.ascontiguousarray(rp.reshape(RT, P, 1)),
                "emb": emb,
                "pemb": pemb,
                "wt": wt,
                "bias": bias,
            }
        )
    return in_maps


def run(inputs, trace=False):
    nc = build_nc()
    in_maps = _prep_inputs(inputs)
    res = run_bass_kernel_spmd(nc, in_maps, list(range(NCORES)), trace=trace)
    out = np.concatenate([res.results[i]["out"] for i in range(NCORES)], axis=0)
    return out, res


def kernel(**inputs):
    out, _ = run(inputs, trace=False)
    return out


# revision 6
# speedup vs baseline: 1.7957x; 1.0562x over previous
"""Trainium2 Bass kernel for nn_AttentiveEncoderPOS — v8.

Sequence-parallel fp8 DoubleRow attention, 2 collectives, restructured for
minimal tensor-engine idle:
  - Phase 1 pipelines per row-tile: gather -> cast -> 16 transposes into a
    k-major staging tile -> strided drain; the linear reads contiguous
    512-row slices. A warmup transpose burst un-throttles the PE clock
    (HAM) while the first gathers are in flight.
  - Collective 1: L.T row-half 0. Collective 2: L.T row-half 1 + V
    (combined; each op pays ~35us fixed rendezvous).
  - Phase 2: scores for ALL 8 key blocks first (exp tiles for the whole
    row stay in SBUF, ~16 MB), then A@V as 64-matmul PSUM accumulation
    chains per query tile — no intermediate output accumulation in SBUF,
    and the V data arrives long before the first A@V needs it.
  - Softmax denominator: vector-engine partial sums of exp tiles, reduced
    by per-tile transpose + free-axis reduce at the end.
"""

import numpy as np

import concourse.bass as bass
import concourse.mybir as mybir
from concourse import bacc
from concourse.tile import TileContext
from concourse.bass_utils import run_bass_kernel_spmd
from concourse.masks import make_identity

N = 8192
H = 1024
VOCAB = 50257
POS = 64
NCORES = 8
NL = N // NCORES          # 1024 rows per core
P = 128
RT = NL // P              # 8 row tiles per core
HT = H // P               # 8 h tiles
GT = HT // 2              # 4 h-tile pairs (DoubleRow)
K2 = 2 * H
KTI = K2 // P             # 16 contraction tiles for the linear
HC = NL // 2              # 512-row halves for the split AllGather
SCALE = 1.0 / 32.0        # 1/sqrt(H)
WARMUP = 56               # PE warmup transposes (~3.4us busy to lift HAM)

BF = mybir.dt.bfloat16
F32 = mybir.dt.float32
F8 = mybir.dt.float8e4
I32 = mybir.dt.int32
EXP = mybir.ActivationFunctionType.Exp
DR = mybir.MatmulPerfMode.DoubleRow


def pair(t, j2=2):
    """[128, 2*F] tile/AP -> [128, 2, F] access pattern for DoubleRow."""
    return t[:].rearrange("p (j f) -> p j f", j=j2)


def build_nc():
    nc = bacc.Bacc(num_devices=NCORES)
    ids = nc.declare_dram_parameter("ids", [RT, P, 1], I32, isOutput=False)
    pids = nc.declare_dram_parameter("pids", [RT, P, 1], I32, isOutput=False)
    emb = nc.declare_dram_parameter("emb", [VOCAB, H], F32, isOutput=False)
    pemb = nc.declare_dram_parameter("pemb", [POS, H], F32, isOutput=False)
    wt = nc.declare_dram_parameter("wt", [K2, H], F32, isOutput=False)  # W.T
    bias = nc.declare_dram_parameter("bias", [HT, P, 1], F32, isOutput=False)
    out = nc.declare_dram_parameter("out", [NL, H], F32, isOutput=True)

    lt_in = [nc.dram_tensor(f"lt_in{h}", [HT, P, HC], F8) for h in range(2)]
    lt_all = [
        nc.dram_tensor(f"lt_all{h}", [NCORES, HT, P, HC], F8, addr_space="Shared")
        for h in range(2)
    ]
    v_in = nc.dram_tensor("v_in", [RT, P, H], F8)
    v_all = nc.dram_tensor("v_all", [NCORES, RT, P, H], F8, addr_space="Shared")
    groups = [list(range(NCORES))]

    with TileContext(nc) as tc:
        with (
            tc.tile_pool(name="const", bufs=1) as const,
            tc.tile_pool(name="persist", bufs=1) as persist,
        ):
            ident8 = const.tile([P, P], F8)
            make_identity(nc, ident8[:])
            ident32 = const.tile([P, P], F32)
            make_identity(nc, ident32[:])
            ident = const.tile([P, P], BF)
            make_identity(nc, ident[:])
            b_sb = const.tile([P, HT], F32)
            nc.sync.dma_start(
                out=b_sb[:].rearrange("p (h u) -> p h u", h=HT),
                in_=bias.rearrange("h p u -> p h u"),
            )
            warm_sb = const.tile([P, P], BF)

            q8 = [persist.tile([P, 2 * NL], F8, name=f"q8{g}") for g in range(GT)]
            cs_acc = persist.tile([P, NL], F32, name="cs_acc")
            nc.gpsimd.memset(cs_acc[:], 0.0)

            # ---------------- Phase 1 ----------------
            with (
                tc.tile_pool(name="wtp", bufs=KTI) as wtp,
                tc.tile_pool(name="wld", bufs=2) as wld,
                tc.tile_pool(name="idp", bufs=2 * RT) as idp,
                tc.tile_pool(name="xfp", bufs=3) as xfp,
                tc.tile_pool(name="xbp", bufs=4) as xbp,
                tc.tile_pool(name="xtr", bufs=2) as xtrp,
                tc.tile_pool(name="tps", bufs=2, space="PSUM") as tps,
                tc.tile_pool(name="mps", bufs=3, space="PSUM") as mps,
                tc.tile_pool(name="wps", bufs=1, space="PSUM") as wps,
            ):
                # PE warmup: lift the HAM clock gate before real work lands
                wp = wps.tile([P, P], BF, tag="w")
                for i in range(WARMUP):
                    nc.tensor.transpose(wp[:], ident[:], ident[:])
                nc.vector.tensor_copy(out=warm_sb[:], in_=wp[:])

                idts, pidts = [], []
                for rt in range(RT):
                    idt = idp.tile([P, 1], I32, tag="id")
                    nc.sync.dma_start(out=idt[:], in_=ids[rt])
                    idts.append(idt)
                    pidt = idp.tile([P, 1], I32, tag="pid")
                    nc.sync.dma_start(out=pidt[:], in_=pids[rt])
                    pidts.append(pidt)

                wtb = []
                for rc in range(2):
                    # X.T staging for this half, k-major: [128, (k, r, c)]
                    xtr = xtrp.tile([P, KTI * HC], BF, tag="xt")
                    for i in range(4):
                        rt = rc * 4 + i
                        xf = xfp.tile([P, K2], F32, tag="xf")
                        nc.gpsimd.indirect_dma_start(
                            out=xf[:, 0:H],
                            out_offset=None,
                            in_=emb[:],
                            in_offset=bass.IndirectOffsetOnAxis(
                                ap=idts[rt][:, :1], axis=0
                            ),
                        )
                        nc.gpsimd.indirect_dma_start(
                            out=xf[:, H:K2],
                            out_offset=None,
                            in_=pemb[:],
                            in_offset=bass.IndirectOffsetOnAxis(
                                ap=pidts[rt][:, :1], axis=0
                            ),
                        )
                        xb = xbp.tile([P, K2], BF, tag="xb")
                        nc.vector.tensor_copy(out=xb[:], in_=xf[:])
                        # 16 transposes of this row tile, k-major in PSUM
                        pt = tps.tile([P, KTI * P], BF, tag="tp")
                        for k in range(KTI):
                            nc.tensor.transpose(
                                pt[:, k * P : (k + 1) * P],
                                xb[:, k * P : (k + 1) * P],
                                ident[:],
                            )
                        nc.vector.tensor_copy(
                            out=xtr[:]
                            .rearrange("p (k r c) -> p k r c", k=KTI, r=4)[
                                :, :, i : i + 1, :
                            ],
                            in_=pt[:].rearrange("p (k c) -> p k c", k=KTI),
                        )

                    if rc == 0:
                        # W.T loads ride the sync queue; casts on the scalar
                        # engine (vector is busy with X)
                        for k in range(KTI):
                            wf = wld.tile([P, H], F32, tag="wld")
                            nc.sync.dma_start(
                                out=wf[:], in_=wt[k * P : (k + 1) * P, :]
                            )
                            wb = wtp.tile([P, H], BF, tag="wtb")
                            if k < 6:
                                nc.vector.tensor_copy(out=wb[:], in_=wf[:])
                            else:
                                nc.scalar.copy(out=wb[:], in_=wf[:])
                            wtb.append(wb)

                    # linear for this half; L.T to fp8 pair layout
                    for ht in range(HT):
                        pm = mps.tile([P, HC], F32, tag="mp")
                        for k in range(KTI):
                            nc.tensor.matmul(
                                pm[:],
                                lhsT=wtb[k][:, ht * P : (ht + 1) * P],
                                rhs=xtr[:, k * HC : (k + 1) * HC],
                                start=(k == 0),
                                stop=(k == KTI - 1),
                            )
                        g, j = ht // 2, ht % 2
                        nc.vector.tensor_add(
                            out=q8[g][:, j * NL + rc * HC : j * NL + (rc + 1) * HC],
                            in0=pm[:],
                            in1=b_sb[:, ht : ht + 1].to_broadcast([P, HC]),
                        )
                        nc.sync.dma_start(
                            out=lt_in[rc][ht],
                            in_=q8[g][:, j * NL + rc * HC : j * NL + (rc + 1) * HC],
                        )
                    nc.gpsimd.collective_compute(
                        "AllGather",
                        mybir.AluOpType.bypass,
                        replica_groups=groups,
                        ins=[lt_in[rc][:].opt()],
                        outs=[lt_all[rc][:].opt()],
                    )

                # own V tiles -> combined collective input
                for rt in range(RT):
                    ptv = tps.tile([P, KTI * P], BF, tag="tp")
                    pv8 = ptv[:].bitcast(F8)
                    pst = pv8.rearrange("p (c two) -> p c two", two=2)
                    for ht in range(HT):
                        g, jj = ht // 2, ht % 2
                        nc.tensor.transpose(
                            pst[:, ht * P : (ht + 1) * P, 0:1],
                            q8[g][:, jj * NL + rt * P : jj * NL + (rt + 1) * P],
                            ident8[:],
                        )
                    vo = xbp.tile([P, H], F8, tag="vo")
                    nc.scalar.copy(out=vo[:], in_=pst[:, 0:H, 0:1])
                    nc.sync.dma_start(out=v_in[rt], in_=vo[:])
                nc.gpsimd.collective_compute(
                    "AllGather",
                    mybir.AluOpType.bypass,
                    replica_groups=groups,
                    ins=[v_in[:].opt()],
                    outs=[v_all[:].opt()],
                )

            # ---------------- Phase 2 ----------------
            with (
                tc.tile_pool(name="lkp", bufs=4 * GT + 4) as lkp,
                tc.tile_pool(name="vp2", bufs=4 * NCORES + 2) as vp2,
                tc.tile_pool(name="ep", bufs=4 * NCORES + 2) as ep,
                tc.tile_pool(name="fin", bufs=2) as fin,
            ):
              with (
                tc.tile_pool(name="sps", bufs=4, space="PSUM") as sps,
                tc.tile_pool(name="ops", bufs=2, space="PSUM") as ops,
              ):
                es = [[None] * 4 for _ in range(NCORES)]
                vts = [[None] * 4 for _ in range(NCORES)]
                # all scores first, half 0 of every block then half 1
                for half in range(2):
                    for b in range(NCORES):
                        ltc = []
                        for g in range(GT):
                            t = lkp.tile([P, 2 * HC], F8, tag="lk")
                            src0 = lt_all[half][b]
                            nc.sync.dma_start(out=t[:, 0:HC], in_=src0[2 * g])
                            nc.sync.dma_start(
                                out=t[:, HC : 2 * HC], in_=src0[2 * g + 1]
                            )
                            ltc.append(t)
                        for ktp in range(2):
                            m = half * 2 + ktp
                            vt = vp2.tile([P, 2 * H], F8, tag="v2")
                            nc.gpsimd.dma_start(
                                out=vt[:, 0:H], in_=v_all[b, 2 * m]
                            )
                            nc.gpsimd.dma_start(
                                out=vt[:, H : 2 * H], in_=v_all[b, 2 * m + 1]
                            )
                            vts[b][m] = vt
                        for ktp in range(2):
                            e = ep.tile([P, 2 * NL], F8, tag="e")
                            for j in range(2):
                                kt = ktp * 2 + j
                                for qc in range(NL // 512):
                                    ps = sps.tile([P, 512], F32, tag="sp")
                                    for g in range(GT):
                                        nc.tensor.matmul(
                                            ps[:],
                                            lhsT=pair(ltc[g])[
                                                :, :, kt * P : (kt + 1) * P
                                            ],
                                            rhs=pair(q8[g])[
                                                :, :, qc * 512 : (qc + 1) * 512
                                            ],
                                            start=(g == 0),
                                            stop=(g == GT - 1),
                                            perf_mode=DR,
                                        )
                                    nc.scalar.activation(
                                        out=e[:, j * NL + qc * 512 : j * NL + (qc + 1) * 512],
                                        in_=ps[:],
                                        func=EXP,
                                        scale=SCALE,
                                    )
                                nc.vector.tensor_add(
                                    out=cs_acc[:],
                                    in0=cs_acc[:],
                                    in1=e[:, j * NL : (j + 1) * NL],
                                )
                            es[b][half * 2 + ktp] = e

                # denominator -> per-query reciprocal (overlaps with A@V)
                den = fin.tile([P, RT], F32, tag="den")
                for qt in range(RT):
                    ct = sps.tile([P, P], F32, tag="sp")
                    nc.tensor.transpose(
                        ct[:], cs_acc[:, qt * P : (qt + 1) * P], ident32[:]
                    )
                    nc.vector.tensor_reduce(
                        out=den[:, qt : qt + 1],
                        in_=ct[:],
                        axis=mybir.AxisListType.X,
                        op=mybir.AluOpType.add,
                    )
                rec = fin.tile([P, RT], F32, tag="rec")
                nc.vector.reciprocal(rec[:], den[:])

                # A@V: one 64-matmul chain per (query tile, h half) in PSUM
                for qt in range(RT):
                    po = ops.tile([P, H], F32, tag="op")
                    for mm in range(4 * NCORES):
                        b, m = mm // 4, mm % 4
                        for hh in range(H // 512):
                            nc.tensor.matmul(
                                po[:, hh * 512 : (hh + 1) * 512],
                                lhsT=pair(es[b][m])[:, :, qt * P : (qt + 1) * P],
                                rhs=pair(vts[b][m])[:, :, hh * 512 : (hh + 1) * 512],
                                start=(mm == 0),
                                stop=(mm == 4 * NCORES - 1),
                                perf_mode=DR,
                            )
                    ot = fin.tile([P, H], F32, tag="ot")
                    nc.vector.tensor_mul(
                        out=ot[:],
                        in0=po[:],
                        in1=rec[:, qt : qt + 1].to_broadcast([P, H]),
                    )
                    nc.sync.dma_start(
                        out=out[qt * P : (qt + 1) * P, :], in_=ot[:]
                    )
    nc.finalize()
    return nc


def _prep_inputs(inputs):
    ids = np.asarray(inputs["input_ids"]).astype(np.int32)
    pids = np.asarray(inputs["pos_ids"]).astype(np.int32)
    emb = np.asarray(inputs["emb"], dtype=np.float32)
    pemb = np.asarray(inputs["pos_emb"], dtype=np.float32)
    W = np.asarray(inputs["W"], dtype=np.float32)
    b = np.asarray(inputs["b"], dtype=np.float32)
    wt = np.ascontiguousarray(W.T)                      # [2H, H]
    bias = np.ascontiguousarray(b.reshape(HT, P, 1))
    in_maps = []
    for i in range(NCORES):
        r = ids[i * NL : (i + 1) * NL]
        rp = pids[i * NL : (i + 1) * NL]
        in_maps.append(
            {
                "ids": np.ascontiguousarray(r.reshape(RT, P, 1)),
                "pids": np.ascontiguousarray(rp.reshape(RT, P, 1)),
                "emb": emb,
                "pemb": pemb,
                "wt": wt,
                "bias": bias,
            }
        )
    return in_maps


def run(inputs, trace=False):
    nc = build_nc()
    in_maps = _prep_inputs(inputs)
    res = run_bass_kernel_spmd(nc, in_maps, list(range(NCORES)), trace=trace)
    out = np.concatenate([res.results[i]["out"] for i in range(NCORES)], axis=0)
    return out, res


def kernel(**inputs):
    out, _ = run(inputs, trace=False)
    return out
